# revision 14
# baseline (speedup 1.0000x reference)
"""Trainium2 Bass kernel for nn_BinaryNetFCBlock.

Computes  y = BN(sign(x) @ sign(k))  where
  sign(v) = +1 if v >= 0 else -1            (larq ste_sign forward)
  BN(y)   = (y - moving_mean) * rsqrt(moving_var + 1e-3) + beta

Full shapes: x [8192, 4096] f32, k [4096, 4096] f32, BN params [4096].
Sharding: pure data-parallel on the batch dim across 8 NeuronCores
(1024 rows each); every core consumes the full kernel matrix.

Per-core plan (all compute on device):
  x path:  DMA f32 -> DVE (is_ge 0, sub 0.5) -> +-0.5 fp8 -> DRAM scratch
           laid out block-major [jj, b, 256] so the xbar transpose reads
           contiguously -> DMA-transpose as u16 (fp8 pairs) directly into
           the packed xT tile: partition dp holds d = jj*256 + 2*dp + ko.
  k path:  DMA f32 pair-interleaved (rows 2p,2p+1 per partition) -> ACT
           Sign(x*1e30 + 1) -> +-1 fp8, same d = jj*256 + 2*p + ko map.
  matmul:  fp8 DoubleRow, lhsT = kq [128, 2, 128] (ko step = NGC),
           rhs = xT [128, 2, 512] (ko step 1, b step 2); PSUM accumulates
           y^T blocks [n_tile=128, b=1024] over K=4096 (16 DR steps).
  epilog:  one DVE tensor_scalar: out = psum * s_eff[n] + t[n]  (s,t are
           per-partition vectors because the psum partition dim is n)
           s_eff = 2 * rsqrt(var+eps)  (2 compensates the +-0.5 x code)
           t     = beta - mean * rsqrt(var+eps)
  output:  y^T [4096, 1024] f32 per core; host transposes + concatenates.
"""

import sys

for _p in ("/opt/trn_rl_repo",):
    if _p not in sys.path:
        sys.path.append(_p)

import contextlib

import numpy as np

import concourse.bass as bass
import concourse.mybir as mybir
import concourse.tile as tile
from concourse import bacc

F32 = mybir.dt.float32
BF16 = mybir.dt.bfloat16
FP8 = mybir.dt.float8e4
U16 = mybir.dt.uint16
AF = mybir.ActivationFunctionType
ALU = mybir.AluOpType
DR = mybir.MatmulPerfMode.DoubleRow

BN_EPS = 1e-3
# ACT Sign computes sign(in*scale + bias).  The scale blows tiny-but-normal
# inputs up to a comfortably normal range, and bias=+1 maps in==0 to +1
# (matching where(x>=0, 1, -1)): |x| >= ~1e-7 for randn-derived inputs, so
# x*1e30 dominates the +1.
SIGN_SCALE = 1e30
SIGN_BIAS = 1.0

P = 128


def emit_kernel(tc, outs, ins, cfg):
    """Emit the per-core tile kernel. outs/ins are dicts of bass.APs."""
    nc = tc.nc
    BS, D, N = cfg["BS"], cfg["D"], cfg["N"]

    x_ap = ins["input_tensor"]
    k_ap = ins["kernel"]
    beta_ap = ins["beta"]
    mean_ap = ins["moving_mean"]
    var_ap = ins["moving_var"]
    yT_ap = outs["outT"]

    NJJ = D // (2 * P)    # 16 blocks of 256 contraction rows (1 DR step each)
    NT = N // P           # 32 output n-tiles (psum partition dim)
    BC = min(512, BS)     # moving-operand b chunk (psum bank = 512 f32)
    NB = BS // BC         # b chunks per psum tile
    G = cfg.get("G", 2)   # n-tiles per kq residency group
    NGC = G * P           # n columns per group
    NGRP = NT // G
    JC = min(cfg.get("JC", 8), NJJ)   # jj blocks per staged k chunk
    KQS = NJJ // JC
    XC = min(cfg.get("XC", 4096), D)  # x free chunk for load+sign
    NBT = BS // P         # x row tiles
    NJX = XC // (2 * P)   # jj blocks per x chunk

    # fp8 sign scratch, block-major: [jj, b, 256] so one (jj, b-half) is a
    # contiguous region for the u16 xbar transpose read.
    xs = nc.dram_tensor("x_sign_scratch", [NJJ, BS, 2 * P], FP8, kind="Internal")
    xs_ap = xs.ap()

    # k pair-interleaved: partition p of block jj holds rows 2p and 2p+1.
    k_view = k_ap.rearrange("(jj p two) n -> jj p two n", p=P, two=2)

    hw_rings = [nc.sync, nc.scalar]

    with contextlib.ExitStack() as ctx:
        pool = lambda name, bufs, **kw: ctx.enter_context(
            tc.tile_pool(name=name, bufs=bufs, **kw)
        )
        stp = pool("stp", 1)
        xload = pool("xload", cfg.get("xload_bufs", 3))
        xsign = pool("xsign", cfg.get("xsign_bufs", 4))
        xTp = pool("xT", 1)
        kload = pool("kload", cfg.get("kload_bufs", 3))
        kqp = pool("kq", cfg.get("kq_bufs", 2))
        psum = pool("psum", cfg.get("psum_bufs", 3), space="PSUM")
        osb = pool("osb", cfg.get("osb_bufs", 4))

        # ---- BN parameter prep
        # Layout [128, NT]: column nt holds params for n = nt*128 + partition.
        par_view = lambda ap: ap.rearrange("(nt p) -> p nt", p=P)
        var_sb = stp.tile([P, NT], F32)
        mean_sb = stp.tile([P, NT], F32)
        beta_sb = stp.tile([P, NT], F32)
        nc.sync.dma_start(var_sb[:], par_view(var_ap))
        nc.sync.dma_start(mean_sb[:], par_view(mean_ap))
        nc.sync.dma_start(beta_sb[:], par_view(beta_ap))
        eps_t = stp.tile([P, 1], F32)
        nc.gpsimd.memset(eps_t[:], BN_EPS)
        sq = stp.tile([P, NT], F32)
        nc.scalar.activation(sq[:], var_sb[:], AF.Sqrt, bias=eps_t[:])
        inv = stp.tile([P, NT], F32)
        nc.vector.reciprocal(inv[:], sq[:])
        ms = stp.tile([P, NT], F32)
        nc.vector.tensor_mul(ms[:], mean_sb[:], inv[:])
        t_sb = stp.tile([P, NT], F32)
        nc.vector.tensor_sub(t_sb[:], beta_sb[:], ms[:])
        s_sb = stp.tile([P, NT], F32)
        # x encoded as +-0.5 -> products scaled by 0.5 -> compensate with 2x
        nc.vector.tensor_scalar(s_sb[:], inv[:], 2.0, None, op0=ALU.mult)

        # ---- kq production helper (SWDGE loads + ACT sign) ----
        def produce_kq(ng):
            n0 = ng * NGC
            kq = kqp.tile([P, NJJ, 2, NGC], FP8)
            for s2 in range(KQS):
                kl = kload.tile([P, JC, 2, NGC], F32)
                kv = k_view[s2 * JC : (s2 + 1) * JC, :, :, n0 : n0 + NGC]
                for ko in range(2):
                    nc.gpsimd.dma_start(
                        kl[:, :, ko, :],
                        kv[:, :, ko, :].rearrange("jj p n -> p jj n"),
                    )
                nc.scalar.activation(
                    kq[:, s2 * JC : (s2 + 1) * JC, :, :],
                    kl[:],
                    AF.Sign,
                    bias=SIGN_BIAS,
                    scale=SIGN_SCALE,
                )
            return kq

        PREFETCH = cfg.get("kq_prefetch", 2)
        kq_ready = {}
        for ng in range(min(PREFETCH, NGRP)):
            kq_ready[ng] = produce_kq(ng)

        # ---- x path: sign -> block-major scratch -> u16 transpose into xT
        # Whole rows per load (2 MB DMAs use all 16 SDMA engines); stores on
        # SWDGE; transposes alone on the scalar HWDGE ring.
        # packed xT: free bytes of block jj are (b, ko) pairs; as u16 the
        # transpose writes [128 dp, b] halfwords = fp8 pairs (d=2dp, 2dp+1).
        xT = xTp.tile([P, NJJ, 2 * BS], FP8)
        for c in range(D // XC):
            c0 = c * XC
            for bt in range(NBT):
                r0 = bt * P
                xl = xload.tile([P, XC], F32)
                nc.sync.dma_start(xl[:], x_ap[r0 : r0 + P, c0 : c0 + XC])
                xsg = xsign.tile([P, XC], FP8)
                # (x >= 0) - 0.5  ->  +-0.5 exact in fp8; DVE cmp is exact
                nc.vector.tensor_scalar(
                    xsg[:], xl[:], 0.0, 0.5, op0=ALU.is_ge, op1=ALU.subtract
                )
                jj0 = c0 // (2 * P)
                dst = xs_ap[jj0 : jj0 + NJX, r0 : r0 + P, :].rearrange(
                    "jj b dd -> b jj dd"
                )
                src = xsg[:].rearrange("b (jj dd) -> b jj dd", dd=2 * P)
                nc.gpsimd.dma_start(dst, src)
        for bh in range(NB):
            b0 = bh * BC
            for jj in range(NJJ):
                nc.scalar.dma_start(
                    xT[:, jj, 2 * b0 : 2 * (b0 + BC)].bitcast(U16),
                    xs_ap[jj, b0 : b0 + BC, :].bitcast(U16),
                    transpose=True,
                )

        # ---- matmul + epilogue, grouped by NGC output columns
        for ng in range(NGRP):
            kq = kq_ready.pop(ng)
            if ng + PREFETCH < NGRP:
                kq_ready[ng + PREFETCH] = produce_kq(ng + PREFETCH)
            for g in range(G):
                nt = ng * G + g
                ps = psum.tile([P, BS], F32)
                for jj in range(NJJ):
                    lhsT = kq[:, jj, :, g * P : (g + 1) * P]
                    rhs_j = xT[:, jj, :].rearrange("p (b two) -> p two b", two=2)
                    for bc in range(NB):
                        nc.tensor.matmul(
                            ps[:, bc * BC : (bc + 1) * BC],
                            lhsT,
                            rhs_j[:, :, bc * BC : (bc + 1) * BC],
                            start=(jj == 0),
                            stop=(jj == NJJ - 1),
                            perf_mode=DR,
                        )
                ob = osb.tile([P, BS], F32)
                nc.vector.tensor_scalar(
                    ob[:],
                    ps[:],
                    s_sb[:, nt : nt + 1],
                    t_sb[:, nt : nt + 1],
                    op0=ALU.mult,
                    op1=ALU.add,
                )
                nc.sync.dma_start(yT_ap[nt * P : (nt + 1) * P, :], ob[:])


def build_nc(cfg):
    """Build + compile the Bacc module for one core (SPMD: same for all)."""
    BS, D, N = cfg["BS"], cfg["D"], cfg["N"]
    nc = bacc.Bacc(
        "TRN2", target_bir_lowering=False, debug=False, enable_asserts=True
    )
    ins = {
        "input_tensor": nc.dram_tensor(
            "input_tensor", [BS, D], F32, kind="ExternalInput"
        ).ap(),
        "kernel": nc.dram_tensor("kernel", [D, N], F32, kind="ExternalInput").ap(),
        "beta": nc.dram_tensor("beta", [N], F32, kind="ExternalInput").ap(),
        "moving_mean": nc.dram_tensor(
            "moving_mean", [N], F32, kind="ExternalInput"
        ).ap(),
        "moving_var": nc.dram_tensor(
            "moving_var", [N], F32, kind="ExternalInput"
        ).ap(),
    }
    outs = {
        "outT": nc.dram_tensor("outT", [N, BS], F32, kind="ExternalOutput").ap(),
    }
    with tile.TileContext(nc) as tc:
        emit_kernel(tc, outs, ins, cfg)
    nc.compile()
    return nc


FULL_CFG = dict(BS=1024, D=4096, N=4096)
N_CORES = 8

_cached = {}


def _get_nc(key, cfg):
    if key not in _cached:
        _cached[key] = build_nc(cfg)
    return _cached[key]


def kernel(input_tensor, kernel, beta, moving_mean, moving_var, trace=False):
    from concourse.bass_utils import run_bass_kernel_spmd

    B, D = input_tensor.shape
    N = kernel.shape[1]
    BS = B // N_CORES
    cfg = dict(FULL_CFG, BS=BS, D=D, N=N)
    nc = _get_nc(("full", BS, D, N), cfg)

    kf = np.ascontiguousarray(kernel, dtype=np.float32)
    in_maps = []
    for c in range(N_CORES):
        in_maps.append(
            {
                "input_tensor": np.ascontiguousarray(
                    input_tensor[c * BS : (c + 1) * BS], dtype=np.float32
                ),
                "kernel": kf,
                "beta": np.ascontiguousarray(beta, dtype=np.float32),
                "moving_mean": np.ascontiguousarray(moving_mean, dtype=np.float32),
                "moving_var": np.ascontiguousarray(moving_var, dtype=np.float32),
            }
        )
    kw = {}
    if trace:
        kw["trace_cores"] = list(range(N_CORES))
    res = run_bass_kernel_spmd(
        nc, in_maps, core_ids=list(range(N_CORES)), trace=trace, **kw
    )
    out = np.empty((B, N), dtype=np.float32)
    for c in range(N_CORES):
        out[c * BS : (c + 1) * BS, :] = res.results[c]["outT"].T
    if trace:
        return out, res
    return out
